# revision 37
# baseline (speedup 1.0000x reference)
"""Trainium2 Bass kernel for nn_Attention_57715770523708.

Softmax2d attention: scores = q @ k^T / 8, softmax over the HEAD axis
(axis=1), out = attn @ v.  Returns (out, attn) like the reference.

Sharding: B(2) x Sq(4 chunks of 512) across 8 NeuronCores.  Every core
keeps all 16 heads for its query rows, so the head-axis softmax is fully
local; there are no collectives.

Per-core dataflow (all fp16 compute, fp32 accumulation in PSUM):
  - q,k loaded with SWDGE cast-DMA (f32->f16), transposed on-chip to
    [d, s] layout via the DMA xbar transpose (2 heads packed per 128
    partitions: head pair h0 at partitions 0:64, h1 at 64:128).
  - matmul1 per head pair with tile_position row groups -> PSUM scores.
  - ScalarE exp(0.125*s) -> fp16 e tiles [128q, 16h, 1024k].
  - VectorE pairwise-tree sum over heads -> n, reciprocal_approx_fast,
    broadcast multiply -> attn (fp16, in-place over e).
  - attn written to HBM with cast-DMA (f16->f32), 4KB runs.
  - attn tiles block-transposed (DMA xbar) -> [k, q] chunks feeding
    matmul2 (lhsT = v chunk, rhs = attn^T) accumulating out^T = [d, q]
    in PSUM; copied out via ScalarE and DMA'd as outT [16, 64, 512].
  - Host transposes outT -> [16, 512, 64] during unshard.
"""

import numpy as np

B, H, SQ, SK, D = 2, 16, 2048 // 4, 2048, 64   # per-core shapes (SQ local = 512)
KBLK = 512                                      # k block per softmax group

_nc_cache = {}


def _build_nc(h=H, sq=SQ, sk=SK, kblk=KBLK):
    import concourse.bass as bass
    import concourse.tile as tile
    import concourse.mybir as mybir
    from concourse import bacc

    F16 = mybir.dt.float16
    F32 = mybir.dt.float32
    AF = mybir.ActivationFunctionType

    pairs = h // 2
    qt_n = sq // 128          # q tiles of 128 rows
    kb_n = sk // kblk         # k blocks
    kc_per_kb = kblk // 128   # 128-wide k chunks per block
    sb_per_kb = kblk // 512   # 512-wide matmul1 slices per block
    kc_n = sk // 128          # total k chunks

    nc = bacc.Bacc(None, target_bir_lowering=False)
    q_d = nc.dram_tensor("q", [h, sq, D], F32, kind="ExternalInput")
    k_d = nc.dram_tensor("k", [h, sk, D], F32, kind="ExternalInput")
    v_d = nc.dram_tensor("v", [h, sk, D], F32, kind="ExternalInput")
    attn_d = nc.dram_tensor("attn", [h, sq, sk], F16, kind="ExternalOutput")
    outT_d = nc.dram_tensor("outT", [h, D, sq], F32, kind="ExternalOutput")

    with tile.TileContext(nc) as tc:
        import contextlib
        with contextlib.ExitStack() as ctx:
            persist = ctx.enter_context(tc.tile_pool(name="persist", bufs=1))
            loads = ctx.enter_context(tc.tile_pool(name="loads", bufs=2))
            epool = ctx.enter_context(tc.tile_pool(name="epool", bufs=3))
            spool = ctx.enter_context(tc.tile_pool(name="spool", bufs=1))
            tpool = ctx.enter_context(tc.tile_pool(name="tpool", bufs=16))
            opool = ctx.enter_context(tc.tile_pool(name="opool", bufs=2))
            ps_sc = ctx.enter_context(
                tc.tile_pool(name="ps_sc", bufs=2, space=bass.MemorySpace.PSUM))
            ps_oT = ctx.enter_context(
                tc.tile_pool(name="ps_oT", bufs=2, space=bass.MemorySpace.PSUM))

            # ---------------- Phase A: load + transpose q, k; load v -------
            qT2 = []   # per pair: [128=(hh,d), qt_n, 128] fp16
            kT2 = []   # per pair: [128=(hh,d), kc_n, 128] fp16
            v_sb = []  # per head: [128=k%128, kc_n, 64] fp16
            for hp in range(pairs):
                h0, h1 = 2 * hp, 2 * hp + 1
                qn = loads.tile([128, qt_n, 2, D], F16, tag="qn")
                for hh, hx in ((0, h0), (1, h1)):
                    nc.gpsimd.dma_start(
                        out=qn[:, :, hh, :],
                        in_=q_d[hx].rearrange("(a p) d -> p a d", p=128))
                qt_t = persist.tile([128, qt_n, 128], F16, tag=f"qT{hp}")
                nc.sync.dma_start(
                    out=qt_t[:], in_=qn.rearrange("p a b d -> p (a b d)"),
                    transpose=True)
                qT2.append(qt_t)

                kn = loads.tile([128, kc_n, 2, D], F16, tag="kn")
                for hh, hx in ((0, h0), (1, h1)):
                    nc.gpsimd.dma_start(
                        out=kn[:, :, hh, :],
                        in_=k_d[hx].rearrange("(a p) d -> p a d", p=128))
                kt_t = persist.tile([128, kc_n, 128], F16, tag=f"kT{hp}")
                nc.sync.dma_start(
                    out=kt_t[:], in_=kn.rearrange("p a b d -> p (a b d)"),
                    transpose=True)
                kT2.append(kt_t)
            for hx in range(h):
                vt = persist.tile([128, kc_n, D], F16, tag=f"v{hx}")
                nc.gpsimd.dma_start(
                    out=vt[:], in_=v_d[hx].rearrange("(a p) d -> p a d", p=128))
                v_sb.append(vt)

            # ---------------- Phase B: main loop ---------------------------
            # Software-pipelined emission: every cross-engine sink is
            # deferred one group so its FIFO wait is pre-satisfied:
            #   - matmul2 of group g emitted after front(g+1)
            #   - out^T accumulate of group g emitted after matmul2(g+1)
            #   - attn HBM write of group g emitted inside front(g+1) on
            #     the scalar HWDGE queue
            state = {"oT_acc": None, "pw": None, "pa": None}

            def emit_write(pw):
                e, wqt, wkb = pw
                nc.scalar.dma_start(
                    out=attn_d[:, wqt * 128:(wqt + 1) * 128,
                               wkb * kblk:(wkb + 1) * kblk]
                    .rearrange("a p c -> p a c"),
                    in_=e[:])

            def emit_add(pa):
                aqt, akb, oT_ps = pa
                if akb == 0:
                    oT_acc = opool.tile([128, pairs, 128], F32, tag="oT_acc")
                    state["oT_acc"] = oT_acc
                    nc.vector.tensor_copy(state["oT_acc"][:], oT_ps[:])
                else:
                    nc.vector.tensor_add(
                        state["oT_acc"][:], state["oT_acc"][:], oT_ps[:])
                if akb == kb_n - 1:
                    nc.sync.dma_start(
                        out=outT_d[:, :, aqt * 128:(aqt + 1) * 128]
                        .rearrange("(hp hh) d p -> (hh d) hp p", hh=2),
                        in_=state["oT_acc"][:])

            def front(qt, kb):
                # matmul1 + exp -> e [128q, h, kblk] fp16
                e = epool.tile([128, h, kblk], F16, tag="e")
                for hp in range(pairs):
                    sc = ps_sc.tile([128, 2 * kblk], F32, tag="sc")
                    for hh in (0, 1):
                        lo, hi = hh * 64, (hh + 1) * 64
                        for sb in range(sb_per_kb):
                            nc.tensor.matmul(
                                sc[:, hh * kblk + sb * 512:
                                   hh * kblk + (sb + 1) * 512],
                                qT2[hp][lo:hi, qt, :],
                                kT2[hp][lo:hi,
                                        kb * kc_per_kb + sb * 4:
                                        kb * kc_per_kb + (sb + 1) * 4, :],
                                start=True, stop=True,
                                tile_position=(lo, 0))
                    nc.scalar.activation(
                        e[:, 2 * hp:2 * hp + 2, :].rearrange(
                            "p a b -> p (a b)"),
                        sc[:], AF.Exp, bias=0.0, scale=0.125)

                # previous group's attn write, deps long satisfied
                if state["pw"] is not None:
                    emit_write(state["pw"])

                # head-axis softmax pieces on VectorE
                s1 = spool.tile([128, h // 2, kblk], F16, tag="s1")
                nc.vector.tensor_add(
                    s1[:], e[:, 0:h // 2, :], e[:, h // 2:h, :])
                m = h // 2
                while m > 1:
                    nc.vector.tensor_add(
                        s1[:, 0:m // 2, :], s1[:, 0:m // 2, :],
                        s1[:, m // 2:m, :])
                    m //= 2
                n32 = spool.tile([128, kblk], F32, tag="n32")
                nc.vector.tensor_copy(n32[:], s1[:, 0, :])
                r32 = spool.tile([128, kblk], F32, tag="r32")
                nc.vector.reciprocal_approx_fast(out=r32[:], in_=n32[:])
                r16 = spool.tile([128, kblk], F16, tag="r16")
                nc.vector.tensor_copy(r16[:], r32[:])
                r_b = bass.AP(tensor=r16.tensor, offset=r16.offset,
                              ap=[r16.ap[0], [0, h], r16.ap[1]])
                nc.vector.tensor_mul(e[:], e[:], r_b)  # in-place normalize

                state["pw"] = (e, qt, kb)

                # transpose attn tiles for matmul2 (single engine: the xbar
                # transpose must never run concurrently from two queues)
                ats = []
                for hp in range(pairs):
                    at = tpool.tile([128, 2 * kc_per_kb, 128], F16, tag="at")
                    nc.sync.dma_start(
                        out=at[:], in_=e[:, 2 * hp:2 * hp + 2, :],
                        transpose=True)
                    ats.append(at)
                return (qt, kb, e, ats)

            def back(work):
                qt, kb, e, ats = work
                oT_ps = ps_oT.tile([128, pairs, 128], F32, tag="oT")
                for hp in range(pairs):
                    at = ats[hp]
                    # interleave the two heads' chains: adjacent matmuls hit
                    # different PE column groups and run concurrently
                    for j in range(kc_per_kb):
                        kc = kb * kc_per_kb + j
                        for hh in (0, 1):
                            hx = 2 * hp + hh
                            lo = hh * 64
                            nc.tensor.matmul(
                                oT_ps[lo:lo + 64, hp, :],
                                v_sb[hx][:, kc, :],
                                at[:, hh * kc_per_kb + j, :],
                                start=(j == 0),
                                stop=(j == kc_per_kb - 1),
                                tile_position=(0, lo))
                if state["pa"] is not None:
                    emit_add(state["pa"])
                state["pa"] = (qt, kb, oT_ps)

            pending = None
            for qt in range(qt_n):
                for kb in range(kb_n):
                    work = front(qt, kb)
                    if pending is not None:
                        back(pending)
                    pending = work
            back(pending)
            emit_add(state["pa"])
            emit_write(state["pw"])

    nc.compile()
    return nc


def _get_nc(key=(H, SQ, SK, KBLK)):
    if key not in _nc_cache:
        _nc_cache[key] = _build_nc(*key)
    return _nc_cache[key]


def kernel(q, k, v, feature_size=64):
    from concourse.bass_utils import run_bass_kernel_spmd

    q = np.asarray(q, dtype=np.float32)
    k = np.asarray(k, dtype=np.float32)
    v = np.asarray(v, dtype=np.float32)
    nB, nH, nS, nD = q.shape
    assert (nB, nH, nS, nD) == (2, 16, 2048, 64), q.shape

    nc = _get_nc()
    in_maps = []
    for dev in range(8):
        b, qi = dev // 4, dev % 4
        in_maps.append({
            "q": np.ascontiguousarray(q[b, :, qi * SQ:(qi + 1) * SQ, :]),
            "k": np.ascontiguousarray(k[b]),
            "v": np.ascontiguousarray(v[b]),
        })
    res = run_bass_kernel_spmd(nc, in_maps, core_ids=list(range(8)))

    out = np.empty((2, 16, 2048, 64), np.float32)
    attn = np.empty((2, 16, 2048, 2048), np.float32)
    for dev in range(8):
        b, qi = dev // 4, dev % 4
        r = res.results[dev]
        attn[b, :, qi * SQ:(qi + 1) * SQ, :] = r["attn"]
        out[b, :, qi * SQ:(qi + 1) * SQ, :] = r["outT"].transpose(0, 2, 1)
    return out, attn


# revision 41
# speedup vs baseline: 1.0800x; 1.0800x over previous
"""Trainium2 Bass kernel for nn_Attention_57715770523708.

Softmax2d attention: scores = q @ k^T / 8, softmax over the HEAD axis
(axis=1), out = attn @ v.  Returns (out, attn) like the reference.

Sharding: B(2) x Sq(4 chunks of 512) across 8 NeuronCores.  Every core
keeps all 16 heads for its query rows, so the head-axis softmax is fully
local; there are no collectives.

Per-core dataflow (all fp16 compute, fp32 accumulation in PSUM):
  - q,k loaded with SWDGE cast-DMA (f32->f16), transposed on-chip to
    [d, s] layout via the DMA xbar transpose (2 heads packed per 128
    partitions: head pair h0 at partitions 0:64, h1 at 64:128).
  - matmul1 per head pair with tile_position row groups -> PSUM scores.
  - ScalarE exp(0.125*s) -> fp16 e tiles [128q, 16h, 1024k].
  - VectorE pairwise-tree sum over heads -> n, reciprocal_approx_fast,
    broadcast multiply -> attn (fp16, in-place over e).
  - attn written to HBM with cast-DMA (f16->f32), 4KB runs.
  - attn tiles block-transposed (DMA xbar) -> [k, q] chunks feeding
    matmul2 (lhsT = v chunk, rhs = attn^T) accumulating out^T = [d, q]
    in PSUM; copied out via ScalarE and DMA'd as outT [16, 64, 512].
  - Host transposes outT -> [16, 512, 64] during unshard.
"""

import numpy as np

B, H, SQ, SK, D = 2, 16, 2048 // 4, 2048, 64   # per-core shapes (SQ local = 512)
KBLK = 512                                      # k block per softmax group

_nc_cache = {}


def _build_nc(h=H, sq=SQ, sk=SK, kblk=KBLK):
    import concourse.bass as bass
    import concourse.tile as tile
    import concourse.mybir as mybir
    from concourse import bacc

    F16 = mybir.dt.float16
    F32 = mybir.dt.float32
    AF = mybir.ActivationFunctionType

    pairs = h // 2
    qt_n = sq // 128          # q tiles of 128 rows
    kb_n = sk // kblk         # k blocks
    kc_per_kb = kblk // 128   # 128-wide k chunks per block
    sb_per_kb = kblk // 512   # 512-wide matmul1 slices per block
    kc_n = sk // 128          # total k chunks

    nc = bacc.Bacc(None, target_bir_lowering=False)
    q_d = nc.dram_tensor("q", [h, sq, D], F32, kind="ExternalInput")
    k_d = nc.dram_tensor("k", [h, sk, D], F32, kind="ExternalInput")
    v_d = nc.dram_tensor("v", [h, sk, D], F32, kind="ExternalInput")
    attn_d = nc.dram_tensor("attn", [h, sq, sk], F16, kind="ExternalOutput")
    outT_d = nc.dram_tensor("outT", [h, D, sq], F32, kind="ExternalOutput")

    with tile.TileContext(nc) as tc:
        import contextlib
        with contextlib.ExitStack() as ctx:
            persist = ctx.enter_context(tc.tile_pool(name="persist", bufs=1))
            loads = ctx.enter_context(tc.tile_pool(name="loads", bufs=2))
            epool = ctx.enter_context(tc.tile_pool(name="epool", bufs=3))
            spool = ctx.enter_context(tc.tile_pool(name="spool", bufs=1))
            tpool = ctx.enter_context(tc.tile_pool(name="tpool", bufs=16))
            opool = ctx.enter_context(tc.tile_pool(name="opool", bufs=2))
            ps_sc = ctx.enter_context(
                tc.tile_pool(name="ps_sc", bufs=2, space=bass.MemorySpace.PSUM))
            ps_oT = ctx.enter_context(
                tc.tile_pool(name="ps_oT", bufs=2, space=bass.MemorySpace.PSUM))

            # ---------------- Phase A: load + transpose q, k; load v -------
            qT2 = []   # per pair: [128=(hh,d), qt_n, 128] fp16
            kT2 = []   # per pair: [128=(hh,d), kc_n, 128] fp16
            v_sb = []  # per head: [128=k%128, kc_n, 64] fp16
            for hp in range(pairs):
                h0, h1 = 2 * hp, 2 * hp + 1
                qn = loads.tile([128, qt_n, 2, D], F16, tag="qn")
                for hh, hx in ((0, h0), (1, h1)):
                    nc.gpsimd.dma_start(
                        out=qn[:, :, hh, :],
                        in_=q_d[hx].rearrange("(a p) d -> p a d", p=128))
                qt_t = persist.tile([128, qt_n, 128], F16, tag=f"qT{hp}")
                nc.sync.dma_start(
                    out=qt_t[:], in_=qn.rearrange("p a b d -> p (a b d)"),
                    transpose=True)
                qT2.append(qt_t)

                kn = loads.tile([128, kc_n, 2, D], F16, tag="kn")
                for hh, hx in ((0, h0), (1, h1)):
                    nc.gpsimd.dma_start(
                        out=kn[:, :, hh, :],
                        in_=k_d[hx].rearrange("(a p) d -> p a d", p=128))
                kt_t = persist.tile([128, kc_n, 128], F16, tag=f"kT{hp}")
                nc.sync.dma_start(
                    out=kt_t[:], in_=kn.rearrange("p a b d -> p (a b d)"),
                    transpose=True)
                kT2.append(kt_t)
            for hx in range(h):
                vt = persist.tile([128, kc_n, D], F16, tag=f"v{hx}")
                nc.gpsimd.dma_start(
                    out=vt[:], in_=v_d[hx].rearrange("(a p) d -> p a d", p=128))
                v_sb.append(vt)

            # ---------------- Phase B: main loop ---------------------------
            # Software-pipelined emission: every cross-engine sink is
            # deferred one group so its FIFO wait is pre-satisfied:
            #   - matmul2 of group g emitted after front(g+1)
            #   - out^T accumulate of group g emitted after matmul2(g+1)
            #   - attn HBM write of group g emitted inside front(g+1) on
            #     the scalar HWDGE queue
            state = {"oT_acc": None, "pw": None, "pa": None}

            def emit_write(pw):
                e, wqt, wkb = pw
                nc.sync.dma_start(
                    out=attn_d[:, wqt * 128:(wqt + 1) * 128,
                               wkb * kblk:(wkb + 1) * kblk]
                    .rearrange("a p c -> p a c"),
                    in_=e[:])

            def emit_add(pa):
                aqt, akb, oT_ps = pa
                if akb == 0:
                    oT_acc = opool.tile([128, pairs, 128], F32, tag="oT_acc")
                    state["oT_acc"] = oT_acc
                    nc.vector.tensor_copy(state["oT_acc"][:], oT_ps[:])
                else:
                    nc.vector.tensor_add(
                        state["oT_acc"][:], state["oT_acc"][:], oT_ps[:])
                if akb == kb_n - 1:
                    nc.sync.dma_start(
                        out=outT_d[:, :, aqt * 128:(aqt + 1) * 128]
                        .rearrange("(hp hh) d p -> (hh d) hp p", hh=2),
                        in_=state["oT_acc"][:])

            def front(qt, kb):
                # matmul1 + exp -> e [128q, h, kblk] fp16
                e = epool.tile([128, h, kblk], F16, tag="e")
                for hp in range(pairs):
                    sc = ps_sc.tile([128, 2 * kblk], F32, tag="sc")
                    for hh in (0, 1):
                        lo, hi = hh * 64, (hh + 1) * 64
                        for sb in range(sb_per_kb):
                            nc.tensor.matmul(
                                sc[:, hh * kblk + sb * 512:
                                   hh * kblk + (sb + 1) * 512],
                                qT2[hp][lo:hi, qt, :],
                                kT2[hp][lo:hi,
                                        kb * kc_per_kb + sb * 4:
                                        kb * kc_per_kb + (sb + 1) * 4, :],
                                start=True, stop=True,
                                tile_position=(lo, 0))
                    nc.scalar.activation(
                        e[:, 2 * hp:2 * hp + 2, :].rearrange(
                            "p a b -> p (a b)"),
                        sc[:], AF.Exp, bias=0.0, scale=0.125)

                # head-axis softmax pieces on VectorE
                s1 = spool.tile([128, h // 2, kblk], F16, tag="s1")
                nc.vector.tensor_add(
                    s1[:], e[:, 0:h // 2, :], e[:, h // 2:h, :])
                m = h // 2
                while m > 1:
                    nc.vector.tensor_add(
                        s1[:, 0:m // 2, :], s1[:, 0:m // 2, :],
                        s1[:, m // 2:m, :])
                    m //= 2
                n32 = spool.tile([128, kblk], F32, tag="n32")
                nc.vector.tensor_copy(n32[:], s1[:, 0, :])
                r32 = spool.tile([128, kblk], F32, tag="r32")
                nc.vector.reciprocal_approx_fast(out=r32[:], in_=n32[:])
                r16 = spool.tile([128, kblk], F16, tag="r16")
                nc.vector.tensor_copy(r16[:], r32[:])
                r_b = bass.AP(tensor=r16.tensor, offset=r16.offset,
                              ap=[r16.ap[0], [0, h], r16.ap[1]])
                nc.vector.tensor_mul(e[:], e[:], r_b)  # in-place normalize

                emit_write((e, qt, kb))

                # transpose attn tiles for matmul2 (single engine: the xbar
                # transpose must never run concurrently from two queues)
                ats = []
                for hp in range(pairs):
                    at = tpool.tile([128, 2 * kc_per_kb, 128], F16, tag="at")
                    nc.sync.dma_start(
                        out=at[:], in_=e[:, 2 * hp:2 * hp + 2, :],
                        transpose=True)
                    ats.append(at)
                return (qt, kb, e, ats)

            def back(work):
                qt, kb, e, ats = work
                oT_ps = ps_oT.tile([128, pairs, 128], F32, tag="oT")
                for hp in range(pairs):
                    at = ats[hp]
                    # interleave the two heads' chains: adjacent matmuls hit
                    # different PE column groups and run concurrently
                    for j in range(kc_per_kb):
                        kc = kb * kc_per_kb + j
                        for hh in (0, 1):
                            hx = 2 * hp + hh
                            lo = hh * 64
                            nc.tensor.matmul(
                                oT_ps[lo:lo + 64, hp, :],
                                v_sb[hx][:, kc, :],
                                at[:, hh * kc_per_kb + j, :],
                                start=(j == 0),
                                stop=(j == kc_per_kb - 1),
                                tile_position=(0, lo))
                emit_add((qt, kb, oT_ps))

            pending = None
            for qt in range(qt_n):
                for kb in range(kb_n):
                    work = front(qt, kb)
                    if pending is not None:
                        back(pending)
                    pending = work
            back(pending)

    nc.compile()
    return nc


def _get_nc(key=(H, SQ, SK, KBLK)):
    if key not in _nc_cache:
        _nc_cache[key] = _build_nc(*key)
    return _nc_cache[key]


def kernel(q, k, v, feature_size=64):
    from concourse.bass_utils import run_bass_kernel_spmd

    q = np.asarray(q, dtype=np.float32)
    k = np.asarray(k, dtype=np.float32)
    v = np.asarray(v, dtype=np.float32)
    nB, nH, nS, nD = q.shape
    assert (nB, nH, nS, nD) == (2, 16, 2048, 64), q.shape

    nc = _get_nc()
    in_maps = []
    for dev in range(8):
        b, qi = dev // 4, dev % 4
        in_maps.append({
            "q": np.ascontiguousarray(q[b, :, qi * SQ:(qi + 1) * SQ, :]),
            "k": np.ascontiguousarray(k[b]),
            "v": np.ascontiguousarray(v[b]),
        })
    res = run_bass_kernel_spmd(nc, in_maps, core_ids=list(range(8)))

    out = np.empty((2, 16, 2048, 64), np.float32)
    attn = np.empty((2, 16, 2048, 2048), np.float32)
    for dev in range(8):
        b, qi = dev // 4, dev % 4
        r = res.results[dev]
        attn[b, :, qi * SQ:(qi + 1) * SQ, :] = r["attn"]
        out[b, :, qi * SQ:(qi + 1) * SQ, :] = r["outT"].transpose(0, 2, 1)
    return out, attn


# revision 45
# speedup vs baseline: 1.1296x; 1.0459x over previous
"""Trainium2 Bass kernel for nn_Attention_57715770523708.

Softmax2d attention: scores = q @ k^T / 8, softmax over the HEAD axis
(axis=1), out = attn @ v.  Returns (out, attn) like the reference.

Sharding: B(2) x Sq(4 chunks of 512) across 8 NeuronCores.  Every core
keeps all 16 heads for its query rows, so the head-axis softmax is fully
local; there are no collectives.

Per-core dataflow (all fp16 compute, fp32 accumulation in PSUM):
  - q,k loaded with SWDGE cast-DMA (f32->f16), transposed on-chip to
    [d, s] layout via the DMA xbar transpose (2 heads packed per 128
    partitions: head pair h0 at partitions 0:64, h1 at 64:128).
  - matmul1 per head pair with tile_position row groups -> PSUM scores.
  - ScalarE exp(0.125*s) -> fp16 e tiles [128q, 16h, 1024k].
  - VectorE pairwise-tree sum over heads -> n, reciprocal_approx_fast,
    broadcast multiply -> attn (fp16, in-place over e).
  - attn written to HBM with cast-DMA (f16->f32), 4KB runs.
  - attn tiles block-transposed (DMA xbar) -> [k, q] chunks feeding
    matmul2 (lhsT = v chunk, rhs = attn^T) accumulating out^T = [d, q]
    in PSUM; copied out via ScalarE and DMA'd as outT [16, 64, 512].
  - Host transposes outT -> [16, 512, 64] during unshard.
"""

import numpy as np

B, H, SQ, SK, D = 2, 16, 2048 // 4, 2048, 64   # per-core shapes (SQ local = 512)
KBLK = 512                                      # k block per softmax group

_nc_cache = {}


def _build_nc(h=H, sq=SQ, sk=SK, kblk=KBLK):
    import concourse.bass as bass
    import concourse.tile as tile
    import concourse.mybir as mybir
    from concourse import bacc

    F16 = mybir.dt.float16
    F32 = mybir.dt.float32
    AF = mybir.ActivationFunctionType

    pairs = h // 2
    qt_n = sq // 128          # q tiles of 128 rows
    kb_n = sk // kblk         # k blocks
    kc_per_kb = kblk // 128   # 128-wide k chunks per block
    sb_per_kb = kblk // 512   # 512-wide matmul1 slices per block
    kc_n = sk // 128          # total k chunks

    nc = bacc.Bacc(None, target_bir_lowering=False)
    q_d = nc.dram_tensor("q", [h, sq, D], F32, kind="ExternalInput")
    k_d = nc.dram_tensor("k", [h, sk, D], F32, kind="ExternalInput")
    v_d = nc.dram_tensor("v", [h, sk, D], F32, kind="ExternalInput")
    attn_d = nc.dram_tensor("attn", [h, sq, sk], F16, kind="ExternalOutput")
    outT_d = nc.dram_tensor("outT", [h, D, sq], F32, kind="ExternalOutput")

    with tile.TileContext(nc) as tc:
        import contextlib
        with contextlib.ExitStack() as ctx:
            persist = ctx.enter_context(tc.tile_pool(name="persist", bufs=1))
            loads = ctx.enter_context(tc.tile_pool(name="loads", bufs=2))
            epool = ctx.enter_context(tc.tile_pool(name="epool", bufs=3))
            spool = ctx.enter_context(tc.tile_pool(name="spool", bufs=1))
            tpool = ctx.enter_context(tc.tile_pool(name="tpool", bufs=16))
            opool = ctx.enter_context(tc.tile_pool(name="opool", bufs=2))
            ps_sc = ctx.enter_context(
                tc.tile_pool(name="ps_sc", bufs=3, space=bass.MemorySpace.PSUM))
            ps_oT = ctx.enter_context(
                tc.tile_pool(name="ps_oT", bufs=1, space=bass.MemorySpace.PSUM))

            # ---------------- Phase A: load + transpose q, k; load v -------
            qT2 = []   # per pair: [128=(hh,d), qt_n, 128] fp16
            kT2 = []   # per pair: [128=(hh,d), kc_n, 128] fp16
            v_sb = []  # per head: [128=k%128, kc_n, 64] fp16
            for hp in range(pairs):
                h0, h1 = 2 * hp, 2 * hp + 1
                qn = loads.tile([128, qt_n, 2, D], F16, tag="qn")
                for hh, hx in ((0, h0), (1, h1)):
                    nc.gpsimd.dma_start(
                        out=qn[:, :, hh, :],
                        in_=q_d[hx].rearrange("(a p) d -> p a d", p=128))
                qt_t = persist.tile([128, qt_n, 128], F16, tag=f"qT{hp}")
                nc.sync.dma_start(
                    out=qt_t[:], in_=qn.rearrange("p a b d -> p (a b d)"),
                    transpose=True)
                qT2.append(qt_t)

                kn = loads.tile([128, kc_n, 2, D], F16, tag="kn")
                for hh, hx in ((0, h0), (1, h1)):
                    nc.gpsimd.dma_start(
                        out=kn[:, :, hh, :],
                        in_=k_d[hx].rearrange("(a p) d -> p a d", p=128))
                kt_t = persist.tile([128, kc_n, 128], F16, tag=f"kT{hp}")
                nc.sync.dma_start(
                    out=kt_t[:], in_=kn.rearrange("p a b d -> p (a b d)"),
                    transpose=True)
                kT2.append(kt_t)
            for hx in range(h):
                vt = persist.tile([128, kc_n, D], F16, tag=f"v{hx}")
                nc.gpsimd.dma_start(
                    out=vt[:], in_=v_d[hx].rearrange("(a p) d -> p a d", p=128))
                v_sb.append(vt)

            # ---------------- Phase B: main loop ---------------------------
            # 3-stage software pipeline: per iteration emit
            #   stage A of group g   (matmul1 + exp),
            #   stage B of group g-1 (softmax + HBM write + transposes),
            #   stage C of group g-2 (matmul2 + out^T accumulate).
            # PE then always has stage-C work whose transposes finished a
            # full group earlier, so it never idles long enough for HAM to
            # re-throttle the clock.
            state = {"oT_acc": None}

            def emit_write(pw):
                e, wqt, wkb = pw
                nc.sync.dma_start(
                    out=attn_d[:, wqt * 128:(wqt + 1) * 128,
                               wkb * kblk:(wkb + 1) * kblk]
                    .rearrange("a p c -> p a c"),
                    in_=e[:])

            def emit_add(pa):
                aqt, akb, oT_ps = pa
                if akb == 0:
                    oT_acc = opool.tile([128, pairs, 128], F32, tag="oT_acc")
                    state["oT_acc"] = oT_acc
                    nc.vector.tensor_copy(state["oT_acc"][:], oT_ps[:])
                else:
                    nc.vector.tensor_add(
                        state["oT_acc"][:], state["oT_acc"][:], oT_ps[:])
                if akb == kb_n - 1:
                    nc.sync.dma_start(
                        out=outT_d[:, :, aqt * 128:(aqt + 1) * 128]
                        .rearrange("(hp hh) d p -> (hh d) hp p", hh=2),
                        in_=state["oT_acc"][:])

            def front_a(qt, kb):
                # matmul1 + exp -> e [128q, h, kblk] fp16
                e = epool.tile([128, h, kblk], F16, tag="e")
                for hp in range(pairs):
                    sc = ps_sc.tile([128, 2 * kblk], F32, tag="sc")
                    for hh in (0, 1):
                        lo, hi = hh * 64, (hh + 1) * 64
                        for sb in range(sb_per_kb):
                            nc.tensor.matmul(
                                sc[:, hh * kblk + sb * 512:
                                   hh * kblk + (sb + 1) * 512],
                                qT2[hp][lo:hi, qt, :],
                                kT2[hp][lo:hi,
                                        kb * kc_per_kb + sb * 4:
                                        kb * kc_per_kb + (sb + 1) * 4, :],
                                start=True, stop=True,
                                tile_position=(lo, 0))
                    nc.scalar.activation(
                        e[:, 2 * hp:2 * hp + 2, :].rearrange(
                            "p a b -> p (a b)"),
                        sc[:], AF.Exp, bias=0.0, scale=0.125)
                return (qt, kb, e)

            def front_b(work):
                qt, kb, e = work
                # head-axis softmax pieces on VectorE
                s1 = spool.tile([128, h // 2, kblk], F16, tag="s1")
                nc.vector.tensor_add(
                    s1[:], e[:, 0:h // 2, :], e[:, h // 2:h, :])
                m = h // 2
                while m > 1:
                    nc.vector.tensor_add(
                        s1[:, 0:m // 2, :], s1[:, 0:m // 2, :],
                        s1[:, m // 2:m, :])
                    m //= 2
                n32 = spool.tile([128, kblk], F32, tag="n32")
                nc.vector.tensor_copy(n32[:], s1[:, 0, :])
                r32 = spool.tile([128, kblk], F32, tag="r32")
                nc.vector.reciprocal_approx_fast(out=r32[:], in_=n32[:])
                r16 = spool.tile([128, kblk], F16, tag="r16")
                nc.vector.tensor_copy(r16[:], r32[:])
                r_b = bass.AP(tensor=r16.tensor, offset=r16.offset,
                              ap=[r16.ap[0], [0, h], r16.ap[1]])
                nc.vector.tensor_mul(e[:], e[:], r_b)  # in-place normalize

                emit_write((e, qt, kb))

                # transpose attn tiles for matmul2 (single engine: the xbar
                # transpose must never run concurrently from two queues)
                ats = []
                for hp in range(pairs):
                    at = tpool.tile([128, 2 * kc_per_kb, 128], F16, tag="at")
                    nc.sync.dma_start(
                        out=at[:], in_=e[:, 2 * hp:2 * hp + 2, :],
                        transpose=True)
                    ats.append(at)
                return (qt, kb, e, ats)

            def back(work):
                qt, kb, e, ats = work
                oT_ps = ps_oT.tile([128, pairs, 128], F32, tag="oT")
                for hp in range(pairs):
                    at = ats[hp]
                    # interleave the two heads' chains: adjacent matmuls hit
                    # different PE column groups and run concurrently
                    for j in range(kc_per_kb):
                        kc = kb * kc_per_kb + j
                        for hh in (0, 1):
                            hx = 2 * hp + hh
                            lo = hh * 64
                            nc.tensor.matmul(
                                oT_ps[lo:lo + 64, hp, :],
                                v_sb[hx][:, kc, :],
                                at[:, hh * kc_per_kb + j, :],
                                start=(j == 0),
                                stop=(j == kc_per_kb - 1),
                                tile_position=(0, lo))
                emit_add((qt, kb, oT_ps))

            pend_a = None   # awaiting stage B
            pend_b = None   # awaiting stage C
            groups = [(qt, kb) for qt in range(qt_n) for kb in range(kb_n)]
            for qt, kb in groups:
                new_b = front_b(pend_a) if pend_a is not None else None
                if pend_b is not None:
                    back(pend_b)
                pend_a = front_a(qt, kb)
                pend_b = new_b
            new_b = front_b(pend_a)
            if pend_b is not None:
                back(pend_b)
            back(new_b)

    nc.compile()
    return nc


def _get_nc(key=(H, SQ, SK, KBLK)):
    if key not in _nc_cache:
        _nc_cache[key] = _build_nc(*key)
    return _nc_cache[key]


def kernel(q, k, v, feature_size=64):
    from concourse.bass_utils import run_bass_kernel_spmd

    q = np.asarray(q, dtype=np.float32)
    k = np.asarray(k, dtype=np.float32)
    v = np.asarray(v, dtype=np.float32)
    nB, nH, nS, nD = q.shape
    assert (nB, nH, nS, nD) == (2, 16, 2048, 64), q.shape

    nc = _get_nc()
    in_maps = []
    for dev in range(8):
        b, qi = dev // 4, dev % 4
        in_maps.append({
            "q": np.ascontiguousarray(q[b, :, qi * SQ:(qi + 1) * SQ, :]),
            "k": np.ascontiguousarray(k[b]),
            "v": np.ascontiguousarray(v[b]),
        })
    res = run_bass_kernel_spmd(nc, in_maps, core_ids=list(range(8)))

    out = np.empty((2, 16, 2048, 64), np.float32)
    attn = np.empty((2, 16, 2048, 2048), np.float32)
    for dev in range(8):
        b, qi = dev // 4, dev % 4
        r = res.results[dev]
        attn[b, :, qi * SQ:(qi + 1) * SQ, :] = r["attn"]
        out[b, :, qi * SQ:(qi + 1) * SQ, :] = r["outT"].transpose(0, 2, 1)
    return out, attn


# revision 46
# speedup vs baseline: 1.1353x; 1.0051x over previous
"""Trainium2 Bass kernel for nn_Attention_57715770523708.

Softmax2d attention: scores = q @ k^T / 8, softmax over the HEAD axis
(axis=1), out = attn @ v.  Returns (out, attn) like the reference.

Sharding: B(2) x Sq(4 chunks of 512) across 8 NeuronCores.  Every core
keeps all 16 heads for its query rows, so the head-axis softmax is fully
local; there are no collectives.

Per-core dataflow (all fp16 compute, fp32 accumulation in PSUM):
  - q,k loaded with SWDGE cast-DMA (f32->f16), transposed on-chip to
    [d, s] layout via the DMA xbar transpose (2 heads packed per 128
    partitions: head pair h0 at partitions 0:64, h1 at 64:128).
  - matmul1 per head pair with tile_position row groups -> PSUM scores.
  - ScalarE exp(0.125*s) -> fp16 e tiles [128q, 16h, 1024k].
  - VectorE pairwise-tree sum over heads -> n, reciprocal_approx_fast,
    broadcast multiply -> attn (fp16, in-place over e).
  - attn written to HBM with cast-DMA (f16->f32), 4KB runs.
  - attn tiles block-transposed (DMA xbar) -> [k, q] chunks feeding
    matmul2 (lhsT = v chunk, rhs = attn^T) accumulating out^T = [d, q]
    in PSUM; copied out via ScalarE and DMA'd as outT [16, 64, 512].
  - Host transposes outT -> [16, 512, 64] during unshard.
"""

import numpy as np

B, H, SQ, SK, D = 2, 16, 2048 // 4, 2048, 64   # per-core shapes (SQ local = 512)
KBLK = 512                                      # k block per softmax group

_nc_cache = {}


def _build_nc(h=H, sq=SQ, sk=SK, kblk=KBLK):
    import concourse.bass as bass
    import concourse.tile as tile
    import concourse.mybir as mybir
    from concourse import bacc

    F16 = mybir.dt.float16
    F32 = mybir.dt.float32
    AF = mybir.ActivationFunctionType

    pairs = h // 2
    qt_n = sq // 128          # q tiles of 128 rows
    kb_n = sk // kblk         # k blocks
    kc_per_kb = kblk // 128   # 128-wide k chunks per block
    sb_per_kb = kblk // 512   # 512-wide matmul1 slices per block
    kc_n = sk // 128          # total k chunks

    nc = bacc.Bacc(None, target_bir_lowering=False)
    q_d = nc.dram_tensor("q", [h, sq, D], F32, kind="ExternalInput")
    k_d = nc.dram_tensor("k", [h, sk, D], F32, kind="ExternalInput")
    v_d = nc.dram_tensor("v", [h, sk, D], F32, kind="ExternalInput")
    attn_d = nc.dram_tensor("attn", [h, sq, sk], F16, kind="ExternalOutput")
    outT_d = nc.dram_tensor("outT", [h, D, sq], F32, kind="ExternalOutput")

    with tile.TileContext(nc) as tc:
        import contextlib
        with contextlib.ExitStack() as ctx:
            persist = ctx.enter_context(tc.tile_pool(name="persist", bufs=1))
            loads = ctx.enter_context(tc.tile_pool(name="loads", bufs=2))
            epool = ctx.enter_context(tc.tile_pool(name="epool", bufs=3))
            spool = ctx.enter_context(tc.tile_pool(name="spool", bufs=1))
            tpool = ctx.enter_context(tc.tile_pool(name="tpool", bufs=16))
            opool = ctx.enter_context(tc.tile_pool(name="opool", bufs=2))
            ps_sc = ctx.enter_context(
                tc.tile_pool(name="ps_sc", bufs=3, space=bass.MemorySpace.PSUM))
            ps_oT = ctx.enter_context(
                tc.tile_pool(name="ps_oT", bufs=1, space=bass.MemorySpace.PSUM))

            # ---------------- Phase A: load + transpose q, k; load v -------
            qT2 = []   # per pair: [128=(hh,d), qt_n, 128] fp16
            kT2 = []   # per pair: [128=(hh,d), kc_n, 128] fp16
            v_sb = []  # per head: [128=k%128, kc_n, 64] fp16
            for hp in range(pairs):
                h0, h1 = 2 * hp, 2 * hp + 1
                qn = loads.tile([128, qt_n, 2, D], F16, tag="qn")
                for hh, hx in ((0, h0), (1, h1)):
                    nc.gpsimd.dma_start(
                        out=qn[:, :, hh, :],
                        in_=q_d[hx].rearrange("(a p) d -> p a d", p=128))
                qt_t = persist.tile([128, qt_n, 128], F16, tag=f"qT{hp}")
                nc.sync.dma_start(
                    out=qt_t[:], in_=qn.rearrange("p a b d -> p (a b d)"),
                    transpose=True)
                qT2.append(qt_t)

                kn = loads.tile([128, kc_n, 2, D], F16, tag="kn")
                for hh, hx in ((0, h0), (1, h1)):
                    nc.gpsimd.dma_start(
                        out=kn[:, :, hh, :],
                        in_=k_d[hx].rearrange("(a p) d -> p a d", p=128))
                kt_t = persist.tile([128, kc_n, 128], F16, tag=f"kT{hp}")
                nc.sync.dma_start(
                    out=kt_t[:], in_=kn.rearrange("p a b d -> p (a b d)"),
                    transpose=True)
                kT2.append(kt_t)
            for hx in range(h):
                vt = persist.tile([128, kc_n, D], F16, tag=f"v{hx}")
                nc.gpsimd.dma_start(
                    out=vt[:], in_=v_d[hx].rearrange("(a p) d -> p a d", p=128))
                v_sb.append(vt)

            # ---------------- Phase B: main loop ---------------------------
            # 3-stage software pipeline: per iteration emit
            #   stage A of group g   (matmul1 + exp),
            #   stage B of group g-1 (softmax + HBM write + transposes),
            #   stage C of group g-2 (matmul2 + out^T accumulate).
            # PE then always has stage-C work whose transposes finished a
            # full group earlier, so it never idles long enough for HAM to
            # re-throttle the clock.
            state = {"oT_acc": None}

            def emit_write(pw):
                e, wqt, wkb = pw
                nc.sync.dma_start(
                    out=attn_d[:, wqt * 128:(wqt + 1) * 128,
                               wkb * kblk:(wkb + 1) * kblk]
                    .rearrange("a p c -> p a c"),
                    in_=e[:])

            def emit_add(pa):
                aqt, akb, oT_ps = pa
                if akb == 0:
                    oT_acc = opool.tile([128, pairs, 128], F32, tag="oT_acc")
                    state["oT_acc"] = oT_acc
                    nc.vector.tensor_copy(state["oT_acc"][:], oT_ps[:])
                else:
                    nc.vector.tensor_add(
                        state["oT_acc"][:], state["oT_acc"][:], oT_ps[:])
                if akb == kb_n - 1:
                    nc.sync.dma_start(
                        out=outT_d[:, :, aqt * 128:(aqt + 1) * 128]
                        .rearrange("(hp hh) d p -> (hh d) hp p", hh=2),
                        in_=state["oT_acc"][:])

            def front_a(qt, kb):
                # matmul1 + exp -> e [128q, h, kblk] fp16
                e = epool.tile([128, h, kblk], F16, tag="e")
                for hp in range(pairs):
                    sc = ps_sc.tile([128, 2 * kblk], F32, tag="sc")
                    for hh in (0, 1):
                        lo, hi = hh * 64, (hh + 1) * 64
                        for sb in range(sb_per_kb):
                            nc.tensor.matmul(
                                sc[:, hh * kblk + sb * 512:
                                   hh * kblk + (sb + 1) * 512],
                                qT2[hp][lo:hi, qt, :],
                                kT2[hp][lo:hi,
                                        kb * kc_per_kb + sb * 4:
                                        kb * kc_per_kb + (sb + 1) * 4, :],
                                start=True, stop=True,
                                tile_position=(lo, 0))
                    nc.scalar.activation(
                        e[:, 2 * hp:2 * hp + 2, :].rearrange(
                            "p a b -> p (a b)"),
                        sc[:], AF.Exp, bias=0.0, scale=0.125)
                return (qt, kb, e)

            def front_b(work):
                qt, kb, e = work
                # head-axis softmax pieces on VectorE; first level split in
                # two so summation starts once half the exps have landed
                s1 = spool.tile([128, h // 2, kblk], F16, tag="s1")
                nc.vector.tensor_add(
                    s1[:, 0:h // 4, :], e[:, 0:h // 4, :],
                    e[:, h // 4:h // 2, :])
                nc.vector.tensor_add(
                    s1[:, h // 4:h // 2, :], e[:, h // 2:3 * h // 4, :],
                    e[:, 3 * h // 4:h, :])
                m = h // 2
                while m > 1:
                    nc.vector.tensor_add(
                        s1[:, 0:m // 2, :], s1[:, 0:m // 2, :],
                        s1[:, m // 2:m, :])
                    m //= 2
                n32 = spool.tile([128, kblk], F32, tag="n32")
                nc.vector.tensor_copy(n32[:], s1[:, 0, :])
                r32 = spool.tile([128, kblk], F32, tag="r32")
                nc.vector.reciprocal_approx_fast(out=r32[:], in_=n32[:])
                r16 = spool.tile([128, kblk], F16, tag="r16")
                nc.vector.tensor_copy(r16[:], r32[:])
                r_b = bass.AP(tensor=r16.tensor, offset=r16.offset,
                              ap=[r16.ap[0], [0, h], r16.ap[1]])
                nc.vector.tensor_mul(e[:], e[:], r_b)  # in-place normalize

                emit_write((e, qt, kb))

                # transpose attn tiles for matmul2 (single engine: the xbar
                # transpose must never run concurrently from two queues)
                ats = []
                for hp in range(pairs):
                    at = tpool.tile([128, 2 * kc_per_kb, 128], F16, tag="at")
                    nc.sync.dma_start(
                        out=at[:], in_=e[:, 2 * hp:2 * hp + 2, :],
                        transpose=True)
                    ats.append(at)
                return (qt, kb, e, ats)

            def back(work):
                qt, kb, e, ats = work
                oT_ps = ps_oT.tile([128, pairs, 128], F32, tag="oT")
                for hp in range(pairs):
                    at = ats[hp]
                    # interleave the two heads' chains: adjacent matmuls hit
                    # different PE column groups and run concurrently
                    for j in range(kc_per_kb):
                        kc = kb * kc_per_kb + j
                        for hh in (0, 1):
                            hx = 2 * hp + hh
                            lo = hh * 64
                            nc.tensor.matmul(
                                oT_ps[lo:lo + 64, hp, :],
                                v_sb[hx][:, kc, :],
                                at[:, hh * kc_per_kb + j, :],
                                start=(j == 0),
                                stop=(j == kc_per_kb - 1),
                                tile_position=(0, lo))
                emit_add((qt, kb, oT_ps))

            pend_a = None   # awaiting stage B
            pend_b = None   # awaiting stage C
            groups = [(qt, kb) for qt in range(qt_n) for kb in range(kb_n)]
            for qt, kb in groups:
                new_b = front_b(pend_a) if pend_a is not None else None
                if pend_b is not None:
                    back(pend_b)
                pend_a = front_a(qt, kb)
                pend_b = new_b
            new_b = front_b(pend_a)
            if pend_b is not None:
                back(pend_b)
            back(new_b)

    nc.compile()
    return nc


def _get_nc(key=(H, SQ, SK, KBLK)):
    if key not in _nc_cache:
        _nc_cache[key] = _build_nc(*key)
    return _nc_cache[key]


def kernel(q, k, v, feature_size=64):
    from concourse.bass_utils import run_bass_kernel_spmd

    q = np.asarray(q, dtype=np.float32)
    k = np.asarray(k, dtype=np.float32)
    v = np.asarray(v, dtype=np.float32)
    nB, nH, nS, nD = q.shape
    assert (nB, nH, nS, nD) == (2, 16, 2048, 64), q.shape

    nc = _get_nc()
    in_maps = []
    for dev in range(8):
        b, qi = dev // 4, dev % 4
        in_maps.append({
            "q": np.ascontiguousarray(q[b, :, qi * SQ:(qi + 1) * SQ, :]),
            "k": np.ascontiguousarray(k[b]),
            "v": np.ascontiguousarray(v[b]),
        })
    res = run_bass_kernel_spmd(nc, in_maps, core_ids=list(range(8)))

    out = np.empty((2, 16, 2048, 64), np.float32)
    attn = np.empty((2, 16, 2048, 2048), np.float32)
    for dev in range(8):
        b, qi = dev // 4, dev % 4
        r = res.results[dev]
        attn[b, :, qi * SQ:(qi + 1) * SQ, :] = r["attn"]
        out[b, :, qi * SQ:(qi + 1) * SQ, :] = r["outT"].transpose(0, 2, 1)
    return out, attn


# revision 50
# speedup vs baseline: 1.4312x; 1.2606x over previous
"""Trainium2 Bass kernel for nn_Attention_57715770523708.

Softmax2d attention: scores = q @ k^T / 8, softmax over the HEAD axis
(axis=1), out = attn @ v.  Returns (out, attn) like the reference.

Sharding: B(2) x Sq(4 chunks of 512) across 8 NeuronCores.  Every core
keeps all 16 heads for its query rows, so the head-axis softmax is fully
local; there are no collectives.

Per-core dataflow (all fp16 compute, fp32 accumulation in PSUM):
  - q,k loaded with SWDGE cast-DMA (f32->f16), transposed on-chip to
    [d, s] layout via the DMA xbar transpose (2 heads packed per 128
    partitions: head pair h0 at partitions 0:64, h1 at 64:128).
  - matmul1 per head pair with tile_position row groups -> PSUM scores.
  - ScalarE exp(0.125*s) -> fp16 e tiles [128q, 16h, 1024k].
  - VectorE pairwise-tree sum over heads -> n, reciprocal_approx_fast,
    broadcast multiply -> attn (fp16, in-place over e).
  - attn written to HBM with cast-DMA (f16->f32), 4KB runs.
  - attn tiles block-transposed (DMA xbar) -> [k, q] chunks feeding
    matmul2 (lhsT = v chunk, rhs = attn^T) accumulating out^T = [d, q]
    in PSUM; copied out via ScalarE and DMA'd as outT [16, 64, 512].
  - Host transposes outT -> [16, 512, 64] during unshard.
"""

import numpy as np

B, H, SQ, SK, D = 2, 16, 2048 // 4, 2048, 64   # per-core shapes (SQ local = 512)
KBLK = 512                                      # k block per softmax group

_nc_cache = {}


def _build_nc(h=H, sq=SQ, sk=SK, kblk=KBLK):
    import concourse.bass as bass
    import concourse.tile as tile
    import concourse.mybir as mybir
    from concourse import bacc

    F16 = mybir.dt.float16
    F32 = mybir.dt.float32
    AF = mybir.ActivationFunctionType

    pairs = h // 2
    qt_n = sq // 128          # q tiles of 128 rows
    kb_n = sk // kblk         # k blocks
    kc_per_kb = kblk // 128   # 128-wide k chunks per block
    sb_per_kb = kblk // 512   # 512-wide matmul1 slices per block
    kc_n = sk // 128          # total k chunks

    nc = bacc.Bacc(None, target_bir_lowering=False)
    q_d = nc.dram_tensor("q", [h, sq, D], F32, kind="ExternalInput")
    k_d = nc.dram_tensor("k", [h, sk, D], F32, kind="ExternalInput")
    v_d = nc.dram_tensor("v", [h, sk, D], F32, kind="ExternalInput")
    attn_d = nc.dram_tensor("attn", [h, sq, sk], F16, kind="ExternalOutput")
    outT_d = nc.dram_tensor("outT", [h, D, sq], F32, kind="ExternalOutput")

    with tile.TileContext(nc) as tc:
        import contextlib
        with contextlib.ExitStack() as ctx:
            persist = ctx.enter_context(tc.tile_pool(name="persist", bufs=1))
            loads = ctx.enter_context(tc.tile_pool(name="loads", bufs=2))
            epool = ctx.enter_context(tc.tile_pool(name="epool", bufs=3))
            spool = ctx.enter_context(tc.tile_pool(name="spool", bufs=1))
            tpool = ctx.enter_context(tc.tile_pool(name="tpool", bufs=16))
            opool = ctx.enter_context(tc.tile_pool(name="opool", bufs=2))
            rpool = ctx.enter_context(tc.tile_pool(name="rpool", bufs=2))
            ps_sc = ctx.enter_context(
                tc.tile_pool(name="ps_sc", bufs=2, space=bass.MemorySpace.PSUM))
            ps_scT = ctx.enter_context(
                tc.tile_pool(name="ps_scT", bufs=1,
                             space=bass.MemorySpace.PSUM))
            ps_oT = ctx.enter_context(
                tc.tile_pool(name="ps_oT", bufs=1, space=bass.MemorySpace.PSUM))

            # ---------------- Phase A: load + transpose q, k; load v -------
            qT2 = []   # per pair: [128=(hh,d), qt_n, 128] fp16
            kT2 = []   # per pair: [128=(hh,d), kc_n, 128] fp16
            v_sb = []  # per head: [128=k%128, kc_n, 64] fp16
            for hp in range(pairs):
                h0, h1 = 2 * hp, 2 * hp + 1
                qn = loads.tile([128, qt_n, 2, D], F16, tag="qn")
                for hh, hx in ((0, h0), (1, h1)):
                    nc.gpsimd.dma_start(
                        out=qn[:, :, hh, :],
                        in_=q_d[hx].rearrange("(a p) d -> p a d", p=128))
                qt_t = persist.tile([128, qt_n, 128], F16, tag=f"qT{hp}")
                nc.sync.dma_start(
                    out=qt_t[:], in_=qn.rearrange("p a b d -> p (a b d)"),
                    transpose=True)
                qT2.append(qt_t)

                kn = loads.tile([128, kc_n, 2, D], F16, tag="kn")
                for hh, hx in ((0, h0), (1, h1)):
                    nc.gpsimd.dma_start(
                        out=kn[:, :, hh, :],
                        in_=k_d[hx].rearrange("(a p) d -> p a d", p=128))
                kt_t = persist.tile([128, kc_n, 128], F16, tag=f"kT{hp}")
                nc.sync.dma_start(
                    out=kt_t[:], in_=kn.rearrange("p a b d -> p (a b d)"),
                    transpose=True)
                kT2.append(kt_t)
            for hx in range(h):
                vt = persist.tile([128, kc_n, D], F16, tag=f"v{hx}")
                nc.gpsimd.dma_start(
                    out=vt[:], in_=v_d[hx].rearrange("(a p) d -> p a d", p=128))
                v_sb.append(vt)

            # ---------------- Phase B: main loop ---------------------------
            # 3-stage software pipeline: per iteration emit
            #   stage A of group g   (matmul1 + exp),
            #   stage B of group g-1 (softmax + HBM write + transposes),
            #   stage C of group g-2 (matmul2 + out^T accumulate).
            # PE then always has stage-C work whose transposes finished a
            # full group earlier, so it never idles long enough for HAM to
            # re-throttle the clock.
            state = {"oT_acc": None}

            def emit_write(pw):
                e, wqt, wkb = pw
                nc.sync.dma_start(
                    out=attn_d[:, wqt * 128:(wqt + 1) * 128,
                               wkb * kblk:(wkb + 1) * kblk]
                    .rearrange("a p c -> p a c"),
                    in_=e[:])

            def emit_add(pa):
                aqt, akb, oT_ps = pa
                if akb == 0:
                    oT_acc = opool.tile([128, pairs, 128], F32, tag="oT_acc")
                    state["oT_acc"] = oT_acc
                    nc.vector.tensor_copy(state["oT_acc"][:], oT_ps[:])
                else:
                    nc.vector.tensor_add(
                        state["oT_acc"][:], state["oT_acc"][:], oT_ps[:])
                if akb == kb_n - 1:
                    nc.sync.dma_start(
                        out=outT_d[:, :, aqt * 128:(aqt + 1) * 128]
                        .rearrange("(hp hh) d p -> (hh d) hp p", hh=2),
                        in_=state["oT_acc"][:])

            def front_a(qt, kb):
                # matmul1 + exp -> e [128q, h, kblk] fp16
                e = epool.tile([128, h, kblk], F16, tag="e")
                for hp in range(pairs):
                    sc = ps_sc.tile([128, 2 * kblk], F32, tag="sc")
                    for hh in (0, 1):
                        lo, hi = hh * 64, (hh + 1) * 64
                        for sb in range(sb_per_kb):
                            nc.tensor.matmul(
                                sc[:, hh * kblk + sb * 512:
                                   hh * kblk + (sb + 1) * 512],
                                qT2[hp][lo:hi, qt, :],
                                kT2[hp][lo:hi,
                                        kb * kc_per_kb + sb * 4:
                                        kb * kc_per_kb + (sb + 1) * 4, :],
                                start=True, stop=True,
                                tile_position=(lo, 0))
                    nc.scalar.activation(
                        e[:, 2 * hp:2 * hp + 2, :].rearrange(
                            "p a b -> p (a b)"),
                        sc[:], AF.Exp, bias=0.0, scale=0.125)
                return (qt, kb, e)

            def front_b(work):
                qt, kb, e = work
                # head-axis softmax pieces on VectorE; first level split in
                # two so summation starts once half the exps have landed
                s1 = spool.tile([128, h // 2, kblk], F16, tag="s1")
                nc.vector.tensor_add(
                    s1[:, 0:h // 4, :], e[:, 0:h // 4, :],
                    e[:, h // 4:h // 2, :])
                nc.vector.tensor_add(
                    s1[:, h // 4:h // 2, :], e[:, h // 2:3 * h // 4, :],
                    e[:, 3 * h // 4:h, :])
                m = h // 2
                while m > 1:
                    nc.vector.tensor_add(
                        s1[:, 0:m // 2, :], s1[:, 0:m // 2, :],
                        s1[:, m // 2:m, :])
                    m //= 2
                n32 = spool.tile([128, kblk], F32, tag="n32")
                nc.vector.tensor_copy(n32[:], s1[:, 0, :])
                r32 = spool.tile([128, kblk], F32, tag="r32")
                nc.vector.reciprocal_approx_fast(out=r32[:], in_=n32[:])
                r16 = spool.tile([128, kblk], F16, tag="r16")
                nc.vector.tensor_copy(r16[:], r32[:])
                r_b = bass.AP(tensor=r16.tensor, offset=r16.offset,
                              ap=[r16.ap[0], [0, h], r16.ap[1]])
                nc.vector.tensor_mul(e[:], e[:], r_b)  # in-place normalize

                emit_write((e, qt, kb))

                # r^T for normalizing the transposed copy (small xbar op)
                rT = rpool.tile([128, kc_per_kb, 128], F16, tag="rT")
                nc.sync.dma_start(out=rT[:], in_=r16[:], transpose=True)

                # attn^T built on-chip: recompute scores^T on TensorE from
                # the resident qT/kT (no 33MB xbar transpose stream), exp on
                # ScalarE straight out of PSUM, normalize in place on
                # VectorE with broadcast r^T.
                ats = []
                for hp in range(pairs):
                    scT = ps_scT.tile([128, 2 * kc_per_kb, 128], F32,
                                      tag="scT")
                    for hh in (0, 1):
                        lo, hi = hh * 64, (hh + 1) * 64
                        for j in range(kc_per_kb):
                            nc.tensor.matmul(
                                scT[:, hh * kc_per_kb + j, :],
                                kT2[hp][lo:hi, kb * kc_per_kb + j, :],
                                qT2[hp][lo:hi, qt, :],
                                start=True, stop=True,
                                tile_position=(lo, 0))
                    at = tpool.tile([128, 2 * kc_per_kb, 128], F16, tag="at")
                    nc.scalar.activation(
                        at.rearrange("p a b -> p (a b)"),
                        scT.rearrange("p a b -> p (a b)"),
                        AF.Exp, bias=0.0, scale=0.125)
                    rT_b = bass.AP(tensor=rT.tensor, offset=rT.offset,
                                   ap=[rT.ap[0], [0, 2], rT.ap[1], rT.ap[2]])
                    at4 = at.rearrange("p (a b) c -> p a b c", a=2)
                    nc.vector.tensor_mul(at4, at4, rT_b)
                    ats.append(at)
                return (qt, kb, e, ats)

            def back(work):
                qt, kb, e, ats = work
                oT_ps = ps_oT.tile([128, pairs, 128], F32, tag="oT")
                for hp in range(pairs):
                    at = ats[hp]
                    for hh in (0, 1):
                        hx = 2 * hp + hh
                        lo = hh * 64
                        for j in range(kc_per_kb):
                            kc = kb * kc_per_kb + j
                            nc.tensor.matmul(
                                oT_ps[lo:lo + 64, hp, :],
                                v_sb[hx][:, kc, :],
                                at[:, hh * kc_per_kb + j, :],
                                start=(j == 0),
                                stop=(j == kc_per_kb - 1),
                                tile_position=(0, lo))
                emit_add((qt, kb, oT_ps))

            pend_a = None   # awaiting stage B
            pend_b = None   # awaiting stage C
            groups = [(qt, kb) for qt in range(qt_n) for kb in range(kb_n)]
            for qt, kb in groups:
                new_b = front_b(pend_a) if pend_a is not None else None
                if pend_b is not None:
                    back(pend_b)
                pend_a = front_a(qt, kb)
                pend_b = new_b
            new_b = front_b(pend_a)
            if pend_b is not None:
                back(pend_b)
            back(new_b)

    nc.compile()
    return nc


def _get_nc(key=(H, SQ, SK, KBLK)):
    if key not in _nc_cache:
        _nc_cache[key] = _build_nc(*key)
    return _nc_cache[key]


def kernel(q, k, v, feature_size=64):
    from concourse.bass_utils import run_bass_kernel_spmd

    q = np.asarray(q, dtype=np.float32)
    k = np.asarray(k, dtype=np.float32)
    v = np.asarray(v, dtype=np.float32)
    nB, nH, nS, nD = q.shape
    assert (nB, nH, nS, nD) == (2, 16, 2048, 64), q.shape

    nc = _get_nc()
    in_maps = []
    for dev in range(8):
        b, qi = dev // 4, dev % 4
        in_maps.append({
            "q": np.ascontiguousarray(q[b, :, qi * SQ:(qi + 1) * SQ, :]),
            "k": np.ascontiguousarray(k[b]),
            "v": np.ascontiguousarray(v[b]),
        })
    res = run_bass_kernel_spmd(nc, in_maps, core_ids=list(range(8)))

    out = np.empty((2, 16, 2048, 64), np.float32)
    attn = np.empty((2, 16, 2048, 2048), np.float32)
    for dev in range(8):
        b, qi = dev // 4, dev % 4
        r = res.results[dev]
        attn[b, :, qi * SQ:(qi + 1) * SQ, :] = r["attn"]
        out[b, :, qi * SQ:(qi + 1) * SQ, :] = r["outT"].transpose(0, 2, 1)
    return out, attn
